# revision 42
# baseline (speedup 1.0000x reference)
"""Trainium2 Bass kernel for NodeEmbeddingLayer (gnn_message_passing).

Math (reference):
    xt = x @ Wn.T + bn                       # [N, H]
    ct = einsum('nkf,hf->nkh', ctx, Wc) + bc # [N, K, H]
    ca = (ct * attn[..,None]).mean(1)        # [N, H]
    u  = (xt + ca) @ Wu.T + bu               # [N, O]
    out = silu(u) @ Wb.T + einsum('nic,oic->no', bspline(u), Ws*scal)

Key rewrites (exact up to fp reassociation):
  1. Linearity of Wc:   ca = (mean_k attn_k*ctx_k) @ Wc.T + bc * (mean_k attn_k)
     -> the [N,K,F]x[H,F] einsum (84 GFLOP) becomes a [N,F] weighted
        reduction (cheap, DVE) + one [N,F]@[F,H] matmul.
  2. Quadratic B-splines on a uniform grid == alternating sums of shifted
     truncated powers relu(y-m)^2, y = 1.5*u + 3.5, m = 0..7:
        B2_c(y) = 0.5*(y-c)+^2 - 1.5*(y-c-1)+^2 + 1.5*(y-c-2)+^2 - 0.5*(y-c-3)+^2
     The (c->m) combination is linear, so it folds into the spline weights
     on the host:  spline = sum_{i,m} relu(y_i-m)^2 * Wt[o,i,m].

Device layout: everything runs transposed ([feature, node]) so that matmul
contractions chain without intermediate transposes; only cmean (produced
node-major by the attention reduction) is PE-transposed on chip. x is
pre-transposed on the host; the output is produced as outT [O, N] and
transposed back on the host.

Sharding: data-parallel over nodes, 2560 nodes/core x 8 cores (core 7 is
zero-padded 2080->2560).
"""

import numpy as np
from contextlib import ExitStack

import concourse.bass as bass
import concourse.tile as tile
from concourse import bacc, mybir, masks
from concourse.bass_utils import run_bass_kernel_spmd

N, K, F, H, O = 20000, 16, 256, 512, 256
NCORES = 8
NPC = 2560              # nodes per core (core 7 zero-padded)
GSZ = 512               # nodes per matmul group (moving free dim)
NG = NPC // GSZ         # 5 groups
NSUB = GSZ // 128       # 4 subtiles of 128 nodes per group
F32 = mybir.dt.float32

import os
USE_F32R = os.environ.get("K_F32R", "1") == "1"
MMDT = mybir.dt.float32r if USE_F32R else mybir.dt.float32

AF = mybir.ActivationFunctionType

# results of the last run_bass_kernel_spmd call (for test harness probing)
last_results = None


def _mm(ap):
    return ap.bitcast(MMDT)


def _build_nc():
    nc = bacc.Bacc("TRN2", target_bir_lowering=False, debug=False)

    ctx_d = nc.declare_dram_parameter("ctxs", [NPC, K, F], F32, isOutput=False)
    xT_d = nc.declare_dram_parameter("xT", [F, NPC], F32, isOutput=False)
    amean_d = nc.declare_dram_parameter("amean", [1, NPC], F32, isOutput=False)
    wn_d = nc.declare_dram_parameter("WnT", [F, H], F32, isOutput=False)
    wc_d = nc.declare_dram_parameter("WcT", [F, H], F32, isOutput=False)
    wu_d = nc.declare_dram_parameter("WuT", [H, O], F32, isOutput=False)
    kb_d = nc.declare_dram_parameter("KbT", [O, O], F32, isOutput=False)
    wsp_d = nc.declare_dram_parameter("Wsp", [16, 128, O], F32, isOutput=False)
    bn_d = nc.declare_dram_parameter("bn", [H], F32, isOutput=False)
    bu_d = nc.declare_dram_parameter("bu", [O], F32, isOutput=False)
    yb_d = nc.declare_dram_parameter("ybias", [O], F32, isOutput=False)
    w2bc_d = nc.declare_dram_parameter("w2bc", [1, O], F32, isOutput=False)
    out_d = nc.declare_dram_parameter("outT", [O, NPC], F32, isOutput=True)

    with tile.TileContext(nc) as tc, ExitStack() as ctx:
        const = ctx.enter_context(tc.tile_pool(name="const", bufs=1))
        ctxp = ctx.enter_context(tc.tile_pool(name="ctxp", bufs=5))
        cmtp = ctx.enter_context(tc.tile_pool(name="cmtp", bufs=2))
        vsbp = ctx.enter_context(tc.tile_pool(name="vsbp", bufs=6))
        actp = ctx.enter_context(tc.tile_pool(name="actp", bufs=3))
        rtp = ctx.enter_context(tc.tile_pool(name="rtp", bufs=3))
        rp = ctx.enter_context(tc.tile_pool(name="rp", bufs=4))
        outp = ctx.enter_context(tc.tile_pool(name="outp", bufs=3))
        ps_st = ctx.enter_context(tc.tile_pool(name="ps_st", bufs=2, space="PSUM"))
        ps_v = ctx.enter_context(tc.tile_pool(name="ps_v", bufs=2, space="PSUM"))
        ps_u = ctx.enter_context(tc.tile_pool(name="ps_u", bufs=2, space="PSUM"))
        ps_o = ctx.enter_context(tc.tile_pool(name="ps_o", bufs=2, space="PSUM"))

        # ---- constants / weights (loaded once) ----
        ident = const.tile([128, 128], F32)
        masks.make_identity(nc, ident[:])

        negm = const.tile([128, 8], F32)
        for m in range(8):
            nc.gpsimd.memset(negm[:, m:m + 1], float(-m))

        def load_rounded(shape, src_ap, name):
            """DMA fp32 into scratch (borrowing a ctx-pool slab), then
            round-copy into an MMDT tile so the tile has an explicit
            fp32r-rounding producer."""
            t = const.tile(shape, MMDT, name=name)
            if not USE_F32R:
                nc.gpsimd.dma_start(t[:], src_ap)
                return t
            free = 1
            for d in shape[1:]:
                free *= d
            s = ctxp.tile([shape[0], free], F32, tag="ctx", name=f"stg_{name}")
            nc.gpsimd.dma_start(s[:], src_ap)
            t_flat = t[:] if len(shape) == 2 else t[:].rearrange("p a b -> p (a b)")
            nc.vector.tensor_copy(t_flat, s[:])
            return t

        wn = load_rounded([128, 2, H],
                          wn_d.rearrange("(c p) h -> p c h", p=128), "wn")
        wc = load_rounded([128, 2, H],
                          wc_d.rearrange("(c p) h -> p c h", p=128), "wc")
        wu = load_rounded([128, 4, O],
                          wu_d.rearrange("(c p) o -> p c o", p=128), "wu")
        kb = load_rounded([128, 2, O],
                          kb_d.rearrange("(c p) o -> p c o", p=128), "kb")
        wsp = load_rounded([128, 16, O], wsp_d.rearrange("j p o -> p j o"), "wsp")
        bn = const.tile([128, 4], F32)
        nc.gpsimd.dma_start(bn[:], bn_d.rearrange("(c p) -> p c", p=128))
        bu = const.tile([128, 2], F32)
        nc.gpsimd.dma_start(bu[:], bu_d.rearrange("(c p) -> p c", p=128))
        yb = const.tile([128, 2], F32)
        nc.gpsimd.dma_start(yb[:], yb_d.rearrange("(c p) -> p c", p=128))
        w2bc = load_rounded([1, O], w2bc_d[:, :], "w2bc")

        xa = const.tile([128, 2, NPC], MMDT)
        if USE_F32R:
            for fc in range(2):
                sx = ctxp.tile([128, NPC], F32, tag="ctx", name=f"stg_x{fc}")
                nc.gpsimd.dma_start(sx[:], xT_d[fc * 128:(fc + 1) * 128, :])
                nc.vector.tensor_copy(xa[:, fc, :], sx[:])
        else:
            nc.gpsimd.dma_start(xa[:], xT_d.rearrange("(c p) n -> p c n", p=128))
        am = load_rounded([1, NPC], amean_d[:, :], "am")

        ctx_r = ctx_d.rearrange("(t p) k f -> t p k f", p=128)

        for g in range(NG):
            gs, ge = g * GSZ, (g + 1) * GSZ

            # ---- context mean (ctx pre-scaled by attn on host) ----
            # cmeanT[f, n] = sum_k ctxs[n, k, f]: 16 accumulating PE
            # transposes straight into PSUM -> [F, nodes] with no DVE work.
            cmt = [cmtp.tile([128, GSZ], F32, tag=f"cmT{fc}", name=f"cmt{fc}_{g}")
                   for fc in range(2)]
            for t in range(NSUB):
                nt = g * NSUB + t
                ct_t = ctxp.tile([128, K, F], F32, tag="ctx", name=f"ctx_{nt}")
                nc.sync.dma_start(ct_t[:], ctx_r[nt])
                # k in [KP, 16): DVE pairwise tree -> s3 [128, F]
                s1 = cmtp.tile([128, 4, F], F32, tag="tr1", name=f"tr1_{nt}")
                nc.vector.tensor_add(s1[:], ct_t[:, 8:12, :], ct_t[:, 12:16, :])
                s2 = cmtp.tile([128, 2, F], F32, tag="tr2", name=f"tr2_{nt}")
                nc.vector.tensor_add(s2[:], s1[:, 0:2, :], s1[:, 2:4, :])
                s3 = cmtp.tile([128, F], F32, tag="tr3", name=f"tr3_{nt}")
                nc.vector.tensor_add(s3[:], s2[:, 0, :], s2[:, 1, :])
                # k in [0, KP): accumulating PE transposes + one for s3
                for fc in range(2):
                    tp = ps_st.tile([128, 128], F32, tag="st", name=f"st{nt}_{fc}")
                    for k in range(8):
                        nc.tensor.matmul(
                            tp[:], ct_t[:, k, fc * 128:(fc + 1) * 128], ident[:],
                            is_transpose=True, start=(k == 0), stop=False)
                    nc.tensor.matmul(
                        tp[:], s3[:, fc * 128:(fc + 1) * 128], ident[:],
                        is_transpose=True, start=False, stop=True)
                    nc.vector.tensor_copy(
                        cmt[fc][:, t * 128:(t + 1) * 128].bitcast(MMDT), tp[:])

            # ---- stage 1: vT = Wn@xT + Wc@cmeanT (+bn via ACT bias) ----
            vsb = []
            for hc in range(4):
                vt = ps_v.tile([128, GSZ], F32, tag="vT", name=f"vt{g}_{hc}")
                hs = hc * 128
                nc.tensor.matmul(vt[:], _mm(wn[:, 0, hs:hs + 128]),
                                 _mm(xa[:, 0, gs:ge]), start=True, stop=False)
                nc.tensor.matmul(vt[:], _mm(wn[:, 1, hs:hs + 128]),
                                 _mm(xa[:, 1, gs:ge]), start=False, stop=False)
                nc.tensor.matmul(vt[:], _mm(wc[:, 0, hs:hs + 128]),
                                 _mm(cmt[0][:]), start=False, stop=False)
                nc.tensor.matmul(vt[:], _mm(wc[:, 1, hs:hs + 128]),
                                 _mm(cmt[1][:]), start=False, stop=True)
                v = vsbp.tile([128, GSZ], F32, tag="vsb", name=f"v{g}_{hc}")
                nc.scalar.activation(v[:].bitcast(MMDT), vt[:], AF.Identity,
                                     bias=bn[:, hc:hc + 1])
                vsb.append(v)

            # ---- stage 2: uT = Wu@vT + (Wu@bc) x amean ----
            silu, ysb = [], []
            for oc in range(2):
                ut = ps_u.tile([128, GSZ], F32, tag="uT", name=f"ut{g}_{oc}")
                os_ = oc * 128
                for hc in range(4):
                    nc.tensor.matmul(ut[:], _mm(wu[:, hc, os_:os_ + 128]),
                                     _mm(vsb[hc][:]), start=(hc == 0), stop=False)
                nc.tensor.matmul(ut[:], _mm(w2bc[0:1, os_:os_ + 128]),
                                 _mm(am[0:1, gs:ge]), start=False, stop=True)
                s = actp.tile([128, GSZ], F32, tag="silu", name=f"s{g}_{oc}")
                nc.scalar.activation(s[:].bitcast(MMDT), ut[:], AF.Silu,
                                     bias=bu[:, oc:oc + 1])
                y = actp.tile([128, GSZ], F32, tag="y", name=f"y{g}_{oc}")
                nc.scalar.activation(y[:], ut[:], AF.Identity,
                                     bias=yb[:, oc:oc + 1], scale=1.5)
                silu.append(s)
                ysb.append(y)

            # ---- stage 3: outT = Kb@silu + Wsp@relu(y-m)^2 ----
            ot = [ps_o.tile([128, GSZ], F32, tag="outT", name=f"ot{g}_{oc}")
                  for oc in range(2)]
            for oc in range(2):
                os_ = oc * 128
                for ic in range(2):
                    nc.tensor.matmul(ot[oc][:], _mm(kb[:, ic, os_:os_ + 128]),
                                     _mm(silu[ic][:]), start=(ic == 0), stop=False)
            for m in range(8):
                for ic in range(2):
                    # relu(y - m) then square; spread across ACT/DVE/GpSimd
                    idx = m * 2 + ic
                    rt = rtp.tile([128, GSZ], F32, tag="rt", name=f"rt{g}_{m}_{ic}")
                    if idx < 8:
                        nc.scalar.activation(rt[:], ysb[ic][:], AF.Relu,
                                             bias=negm[:, m:m + 1])
                    else:
                        nc.vector.tensor_scalar(
                            rt[:], ysb[ic][:], float(-m), 0.0,
                            mybir.AluOpType.add, mybir.AluOpType.max)
                    r = rp.tile([128, GSZ], F32, tag="r", name=f"r{g}_{m}_{ic}")
                    if idx < 6:
                        nc.vector.tensor_mul(r[:].bitcast(MMDT), rt[:], rt[:])
                    else:
                        nc.gpsimd.tensor_mul(r[:].bitcast(MMDT), rt[:], rt[:])
                    j = m * 2 + ic
                    last = (m == 7 and ic == 1)
                    for oc in range(2):
                        os_ = oc * 128
                        nc.tensor.matmul(ot[oc][:], _mm(wsp[:, j, os_:os_ + 128]),
                                         _mm(r[:]), start=False, stop=last)
            for oc in range(2):
                osb = outp.tile([128, GSZ], F32, tag="osb", name=f"osb{g}_{oc}")
                nc.vector.tensor_copy(osb[:], ot[oc][:])
                # separate DMA queue (SWDGE) so output stores never
                # head-of-line block the in-order ctx load stream on SP
                nc.gpsimd.dma_start(out_d[oc * 128:(oc + 1) * 128, gs:ge], osb[:])

    nc.finalize()
    return nc


_nc_cache = None


def _get_nc():
    global _nc_cache
    if _nc_cache is None:
        _nc_cache = _build_nc()
    return _nc_cache


def kernel(x, contexts, attn, Wn, bn, Wc, bc, Wu, bu, kan_base_w,
           kan_spline_w, kan_scaler):
    global last_results
    f32 = np.float32

    x = np.asarray(x, f32)
    contexts = np.asarray(contexts, f32)
    attn = np.asarray(attn, f32)

    # fold spline basis combination into the weights:
    # B2_c(y) = sum_t alpha[t] * relu(y - (c+t))^2, alpha = [.5,-1.5,1.5,-.5]
    ws = np.asarray(kan_spline_w, f32) * np.asarray(kan_scaler, f32)[..., None]
    alpha = np.array([0.5, -1.5, 1.5, -0.5], f32)
    wt = np.zeros((O, O, 8), f32)  # [o, i, m]
    for c in range(5):
        for t in range(4):
            wt[:, :, c + t] += alpha[t] * ws[:, :, c]
    # [(m, i), o] chunked for the PE: j = m*2 + ic, row p = i - ic*128
    wsp_h = np.ascontiguousarray(
        wt.transpose(2, 1, 0).reshape(16, 128, O))

    wn_h = np.ascontiguousarray(np.asarray(Wn, f32).T)       # [F, H]
    wc_h = np.ascontiguousarray(np.asarray(Wc, f32).T)       # [F, H]
    wu_h = np.ascontiguousarray(np.asarray(Wu, f32).T)       # [H, O]
    kb_h = np.ascontiguousarray(np.asarray(kan_base_w, f32).T)  # [O, O]
    bn_h = np.ascontiguousarray(np.asarray(bn, f32))
    bu_h = np.ascontiguousarray(np.asarray(bu, f32))
    yb_h = np.ascontiguousarray(bu_h * f32(1.5) + f32(3.5))
    w2bc_h = np.ascontiguousarray(
        (np.asarray(Wu, f32) @ np.asarray(bc, f32)).reshape(1, O))

    attn_s = attn * f32(1.0 / K)
    amean = attn_s.sum(axis=1, dtype=f32)
    # pre-scale contexts by attention weights (exact same multiply the
    # reference does); on-device the k-reduction becomes a pure sum
    ctx_s = contexts * attn_s[..., None]

    in_maps = []
    for c in range(NCORES):
        s, e = c * NPC, min((c + 1) * NPC, N)
        n = e - s
        xc = x[s:e]
        cc = ctx_s[s:e]
        mc = amean[s:e]
        if n < NPC:
            pad = NPC - n
            xc = np.concatenate([xc, np.zeros((pad, F), f32)])
            cc = np.concatenate([cc, np.zeros((pad, K, F), f32)])
            mc = np.concatenate([mc, np.zeros((pad,), f32)])
        in_maps.append({
            "ctxs": np.ascontiguousarray(cc),
            "xT": np.ascontiguousarray(xc.T),
            "amean": np.ascontiguousarray(mc.reshape(1, NPC)),
            "WnT": wn_h, "WcT": wc_h, "WuT": wu_h, "KbT": kb_h,
            "Wsp": wsp_h, "bn": bn_h, "bu": bu_h, "ybias": yb_h,
            "w2bc": w2bc_h,
        })

    nc = _get_nc()
    res = run_bass_kernel_spmd(nc, in_maps, list(range(NCORES)))
    last_results = res

    out = np.empty((N, O), f32)
    for c in range(NCORES):
        s, e = c * NPC, min((c + 1) * NPC, N)
        out[s:e] = res.results[c]["outT"].T[: e - s]
    return out


# revision 45
# speedup vs baseline: 4.7516x; 4.7516x over previous
"""Trainium2 Bass kernel for NodeEmbeddingLayer (gnn_message_passing).

Math (reference):
    xt = x @ Wn.T + bn                       # [N, H]
    ct = einsum('nkf,hf->nkh', ctx, Wc) + bc # [N, K, H]
    ca = (ct * attn[..,None]).mean(1)        # [N, H]
    u  = (xt + ca) @ Wu.T + bu               # [N, O]
    out = silu(u) @ Wb.T + einsum('nic,oic->no', bspline(u), Ws*scal)

Key rewrites (exact up to fp reassociation):
  1. Linearity of Wc:   ca = (mean_k attn_k*ctx_k) @ Wc.T + bc * (mean_k attn_k)
     -> the [N,K,F]x[H,F] einsum (84 GFLOP) becomes a [N,F] weighted
        reduction (cheap, DVE) + one [N,F]@[F,H] matmul.
  2. Quadratic B-splines on a uniform grid == alternating sums of shifted
     truncated powers relu(y-m)^2, y = 1.5*u + 3.5, m = 0..7:
        B2_c(y) = 0.5*(y-c)+^2 - 1.5*(y-c-1)+^2 + 1.5*(y-c-2)+^2 - 0.5*(y-c-3)+^2
     The (c->m) combination is linear, so it folds into the spline weights
     on the host:  spline = sum_{i,m} relu(y_i-m)^2 * Wt[o,i,m].

Device layout: everything runs transposed ([feature, node]) so that matmul
contractions chain without intermediate transposes; only cmean (produced
node-major by the attention reduction) is PE-transposed on chip. x is
pre-transposed on the host; the output is produced as outT [O, N] and
transposed back on the host.

Sharding: data-parallel over nodes, 2560 nodes/core x 8 cores (core 7 is
zero-padded 2080->2560).
"""

import numpy as np
from contextlib import ExitStack

import concourse.bass as bass
import concourse.tile as tile
from concourse import bacc, mybir, masks
from concourse.bass_utils import run_bass_kernel_spmd

N, K, F, H, O = 20000, 16, 256, 512, 256
NCORES = 8
NPC = 2560              # nodes per core (core 7 zero-padded)
GSZ = 512               # nodes per matmul group (moving free dim)
NG = NPC // GSZ         # 5 groups
NSUB = GSZ // 128       # 4 subtiles of 128 nodes per group
F32 = mybir.dt.float32

import os
USE_F32R = os.environ.get("K_F32R", "1") == "1"
MMDT = mybir.dt.float32r if USE_F32R else mybir.dt.float32

AF = mybir.ActivationFunctionType

# results of the last run_bass_kernel_spmd call (for test harness probing)
last_results = None


def _mm(ap):
    return ap.bitcast(MMDT)


def _build_nc():
    nc = bacc.Bacc("TRN2", target_bir_lowering=False, debug=False)

    ctx_d = nc.declare_dram_parameter("ctxs", [NPC, K, F], F32, isOutput=False)
    xT_d = nc.declare_dram_parameter("xT", [F, NPC], F32, isOutput=False)
    amean_d = nc.declare_dram_parameter("amean", [1, NPC], F32, isOutput=False)
    wn_d = nc.declare_dram_parameter("WnT", [F, H], F32, isOutput=False)
    wc_d = nc.declare_dram_parameter("WcT", [F, H], F32, isOutput=False)
    wu_d = nc.declare_dram_parameter("WuT", [H, O], F32, isOutput=False)
    kb_d = nc.declare_dram_parameter("KbT", [O, O], F32, isOutput=False)
    wsp_d = nc.declare_dram_parameter("Wsp", [16, 128, O], F32, isOutput=False)
    bn_d = nc.declare_dram_parameter("bn", [H], F32, isOutput=False)
    bu_d = nc.declare_dram_parameter("bu", [O], F32, isOutput=False)
    yb_d = nc.declare_dram_parameter("ybias", [O], F32, isOutput=False)
    w2bc_d = nc.declare_dram_parameter("w2bc", [1, O], F32, isOutput=False)
    out_d = nc.declare_dram_parameter("outT", [O, NPC], F32, isOutput=True)

    with tile.TileContext(nc) as tc, ExitStack() as ctx:
        const = ctx.enter_context(tc.tile_pool(name="const", bufs=1))
        ctxp = ctx.enter_context(tc.tile_pool(name="ctxp", bufs=5))
        cmtp = ctx.enter_context(tc.tile_pool(name="cmtp", bufs=2))
        vsbp = ctx.enter_context(tc.tile_pool(name="vsbp", bufs=6))
        actp = ctx.enter_context(tc.tile_pool(name="actp", bufs=3))
        rtp = ctx.enter_context(tc.tile_pool(name="rtp", bufs=3))
        rp = ctx.enter_context(tc.tile_pool(name="rp", bufs=4))
        outp = ctx.enter_context(tc.tile_pool(name="outp", bufs=3))
        ps_st = ctx.enter_context(tc.tile_pool(name="ps_st", bufs=2, space="PSUM"))
        ps_v = ctx.enter_context(tc.tile_pool(name="ps_v", bufs=2, space="PSUM"))
        ps_u = ctx.enter_context(tc.tile_pool(name="ps_u", bufs=2, space="PSUM"))
        ps_o = ctx.enter_context(tc.tile_pool(name="ps_o", bufs=2, space="PSUM"))

        # ---- constants / weights (loaded once) ----
        ident = const.tile([128, 128], F32)
        masks.make_identity(nc, ident[:])

        negm = const.tile([128, 8], F32)
        for m in range(8):
            nc.gpsimd.memset(negm[:, m:m + 1], float(-m))

        def load_rounded(shape, src_ap, name):
            """DMA fp32 into scratch (borrowing a ctx-pool slab), then
            round-copy into an MMDT tile so the tile has an explicit
            fp32r-rounding producer."""
            t = const.tile(shape, MMDT, name=name)
            if not USE_F32R:
                nc.gpsimd.dma_start(t[:], src_ap)
                return t
            free = 1
            for d in shape[1:]:
                free *= d
            s = ctxp.tile([shape[0], free], F32, tag="ctx", name=f"stg_{name}")
            nc.gpsimd.dma_start(s[:], src_ap)
            t_flat = t[:] if len(shape) == 2 else t[:].rearrange("p a b -> p (a b)")
            nc.vector.tensor_copy(t_flat, s[:])
            return t

        wn = load_rounded([128, 2, H],
                          wn_d.rearrange("(c p) h -> p c h", p=128), "wn")
        wc = load_rounded([128, 2, H],
                          wc_d.rearrange("(c p) h -> p c h", p=128), "wc")
        wu = load_rounded([128, 4, O],
                          wu_d.rearrange("(c p) o -> p c o", p=128), "wu")
        kb = load_rounded([128, 2, O],
                          kb_d.rearrange("(c p) o -> p c o", p=128), "kb")
        wsp = load_rounded([128, 16, O], wsp_d.rearrange("j p o -> p j o"), "wsp")
        bn = const.tile([128, 4], F32)
        nc.gpsimd.dma_start(bn[:], bn_d.rearrange("(c p) -> p c", p=128))
        bu = const.tile([128, 2], F32)
        nc.gpsimd.dma_start(bu[:], bu_d.rearrange("(c p) -> p c", p=128))
        yb = const.tile([128, 2], F32)
        nc.gpsimd.dma_start(yb[:], yb_d.rearrange("(c p) -> p c", p=128))
        w2bc = load_rounded([1, O], w2bc_d[:, :], "w2bc")

        xa = const.tile([128, 2, NPC], MMDT)
        if USE_F32R:
            for fc in range(2):
                sx = ctxp.tile([128, NPC], F32, tag="ctx", name=f"stg_x{fc}")
                nc.gpsimd.dma_start(sx[:], xT_d[fc * 128:(fc + 1) * 128, :])
                nc.vector.tensor_copy(xa[:, fc, :], sx[:])
        else:
            nc.gpsimd.dma_start(xa[:], xT_d.rearrange("(c p) n -> p c n", p=128))
        am = load_rounded([1, NPC], amean_d[:, :], "am")

        ctx_r = ctx_d.rearrange("(t p) k f -> t p k f", p=128)

        # last 512-node group split into two 256-node groups to halve
        # the pipeline drain tail
        groups = [(i * GSZ, GSZ) for i in range(NG - 1)]
        groups += [((NG - 1) * GSZ, GSZ // 2), ((NG - 1) * GSZ + GSZ // 2, GSZ // 2)]
        for g, (gs, gsz) in enumerate(groups):
            ge = gs + gsz
            nsub = gsz // 128

            # ---- context mean (ctx pre-scaled by attn on host) ----
            # cmeanT[f, n] = sum_k ctxs[n, k, f]: 16 accumulating PE
            # transposes straight into PSUM -> [F, nodes] with no DVE work.
            cmt = [cmtp.tile([128, gsz], F32, tag=f"cmT{fc}", name=f"cmt{fc}_{g}")
                   for fc in range(2)]
            for t in range(nsub):
                nt = gs // 128 + t
                ct_t = ctxp.tile([128, K, F], F32, tag="ctx", name=f"ctx_{nt}")
                nc.sync.dma_start(ct_t[:], ctx_r[nt])
                # k in [KP, 16): DVE pairwise tree -> s3 [128, F]
                s1 = cmtp.tile([128, 4, F], F32, tag="tr1", name=f"tr1_{nt}")
                nc.vector.tensor_add(s1[:], ct_t[:, 8:12, :], ct_t[:, 12:16, :])
                s2 = cmtp.tile([128, 2, F], F32, tag="tr2", name=f"tr2_{nt}")
                nc.vector.tensor_add(s2[:], s1[:, 0:2, :], s1[:, 2:4, :])
                s3 = cmtp.tile([128, F], F32, tag="tr3", name=f"tr3_{nt}")
                nc.vector.tensor_add(s3[:], s2[:, 0, :], s2[:, 1, :])
                # k in [0, KP): accumulating PE transposes + one for s3
                for fc in range(2):
                    tp = ps_st.tile([128, 128], F32, tag="st", name=f"st{nt}_{fc}")
                    for k in range(8):
                        nc.tensor.matmul(
                            tp[:], ct_t[:, k, fc * 128:(fc + 1) * 128], ident[:],
                            is_transpose=True, start=(k == 0), stop=False)
                    nc.tensor.matmul(
                        tp[:], s3[:, fc * 128:(fc + 1) * 128], ident[:],
                        is_transpose=True, start=False, stop=True)
                    nc.vector.tensor_copy(
                        cmt[fc][:, t * 128:(t + 1) * 128].bitcast(MMDT), tp[:])

            # ---- stage 1: vT = Wn@xT + Wc@cmeanT (+bn via ACT bias) ----
            vsb = []
            for hc in range(4):
                vt = ps_v.tile([128, gsz], F32, tag="vT", name=f"vt{g}_{hc}")
                hs = hc * 128
                nc.tensor.matmul(vt[:], _mm(wn[:, 0, hs:hs + 128]),
                                 _mm(xa[:, 0, gs:ge]), start=True, stop=False)
                nc.tensor.matmul(vt[:], _mm(wn[:, 1, hs:hs + 128]),
                                 _mm(xa[:, 1, gs:ge]), start=False, stop=False)
                nc.tensor.matmul(vt[:], _mm(wc[:, 0, hs:hs + 128]),
                                 _mm(cmt[0][:]), start=False, stop=False)
                nc.tensor.matmul(vt[:], _mm(wc[:, 1, hs:hs + 128]),
                                 _mm(cmt[1][:]), start=False, stop=True)
                v = vsbp.tile([128, gsz], F32, tag="vsb", name=f"v{g}_{hc}")
                nc.scalar.activation(v[:].bitcast(MMDT), vt[:], AF.Identity,
                                     bias=bn[:, hc:hc + 1])
                vsb.append(v)

            # ---- stage 2: uT = Wu@vT + (Wu@bc) x amean ----
            silu, ysb = [], []
            for oc in range(2):
                ut = ps_u.tile([128, gsz], F32, tag="uT", name=f"ut{g}_{oc}")
                os_ = oc * 128
                for hc in range(4):
                    nc.tensor.matmul(ut[:], _mm(wu[:, hc, os_:os_ + 128]),
                                     _mm(vsb[hc][:]), start=(hc == 0), stop=False)
                nc.tensor.matmul(ut[:], _mm(w2bc[0:1, os_:os_ + 128]),
                                 _mm(am[0:1, gs:ge]), start=False, stop=True)
                s = actp.tile([128, gsz], F32, tag="silu", name=f"s{g}_{oc}")
                nc.scalar.activation(s[:].bitcast(MMDT), ut[:], AF.Silu,
                                     bias=bu[:, oc:oc + 1])
                y = actp.tile([128, gsz], F32, tag="y", name=f"y{g}_{oc}")
                nc.scalar.activation(y[:], ut[:], AF.Identity,
                                     bias=yb[:, oc:oc + 1], scale=1.5)
                silu.append(s)
                ysb.append(y)

            # ---- stage 3: outT = Kb@silu + Wsp@relu(y-m)^2 ----
            ot = [ps_o.tile([128, gsz], F32, tag="outT", name=f"ot{g}_{oc}")
                  for oc in range(2)]
            for oc in range(2):
                os_ = oc * 128
                for ic in range(2):
                    nc.tensor.matmul(ot[oc][:], _mm(kb[:, ic, os_:os_ + 128]),
                                     _mm(silu[ic][:]), start=(ic == 0), stop=False)
            for m in range(8):
                for ic in range(2):
                    # relu(y - m) then square; spread across ACT/DVE/GpSimd
                    idx = m * 2 + ic
                    rt = rtp.tile([128, gsz], F32, tag="rt", name=f"rt{g}_{m}_{ic}")
                    if idx < 8:
                        nc.scalar.activation(rt[:], ysb[ic][:], AF.Relu,
                                             bias=negm[:, m:m + 1])
                    else:
                        nc.vector.tensor_scalar(
                            rt[:], ysb[ic][:], float(-m), 0.0,
                            mybir.AluOpType.add, mybir.AluOpType.max)
                    r = rp.tile([128, gsz], F32, tag="r", name=f"r{g}_{m}_{ic}")
                    if idx < 6:
                        nc.vector.tensor_mul(r[:].bitcast(MMDT), rt[:], rt[:])
                    else:
                        nc.gpsimd.tensor_mul(r[:].bitcast(MMDT), rt[:], rt[:])
                    j = m * 2 + ic
                    last = (m == 7 and ic == 1)
                    for oc in range(2):
                        os_ = oc * 128
                        nc.tensor.matmul(ot[oc][:], _mm(wsp[:, j, os_:os_ + 128]),
                                         _mm(r[:]), start=False, stop=last)
            for oc in range(2):
                osb = outp.tile([128, gsz], F32, tag="osb", name=f"osb{g}_{oc}")
                nc.vector.tensor_copy(osb[:], ot[oc][:])
                # separate DMA queue (SWDGE) so output stores never
                # head-of-line block the in-order ctx load stream on SP
                nc.gpsimd.dma_start(out_d[oc * 128:(oc + 1) * 128, gs:ge], osb[:])

    nc.finalize()
    return nc


_nc_cache = None


def _get_nc():
    global _nc_cache
    if _nc_cache is None:
        _nc_cache = _build_nc()
    return _nc_cache


def kernel(x, contexts, attn, Wn, bn, Wc, bc, Wu, bu, kan_base_w,
           kan_spline_w, kan_scaler):
    global last_results
    f32 = np.float32

    x = np.asarray(x, f32)
    contexts = np.asarray(contexts, f32)
    attn = np.asarray(attn, f32)

    # fold spline basis combination into the weights:
    # B2_c(y) = sum_t alpha[t] * relu(y - (c+t))^2, alpha = [.5,-1.5,1.5,-.5]
    ws = np.asarray(kan_spline_w, f32) * np.asarray(kan_scaler, f32)[..., None]
    alpha = np.array([0.5, -1.5, 1.5, -0.5], f32)
    wt = np.zeros((O, O, 8), f32)  # [o, i, m]
    for c in range(5):
        for t in range(4):
            wt[:, :, c + t] += alpha[t] * ws[:, :, c]
    # [(m, i), o] chunked for the PE: j = m*2 + ic, row p = i - ic*128
    wsp_h = np.ascontiguousarray(
        wt.transpose(2, 1, 0).reshape(16, 128, O))

    wn_h = np.ascontiguousarray(np.asarray(Wn, f32).T)       # [F, H]
    wc_h = np.ascontiguousarray(np.asarray(Wc, f32).T)       # [F, H]
    wu_h = np.ascontiguousarray(np.asarray(Wu, f32).T)       # [H, O]
    kb_h = np.ascontiguousarray(np.asarray(kan_base_w, f32).T)  # [O, O]
    bn_h = np.ascontiguousarray(np.asarray(bn, f32))
    bu_h = np.ascontiguousarray(np.asarray(bu, f32))
    yb_h = np.ascontiguousarray(bu_h * f32(1.5) + f32(3.5))
    w2bc_h = np.ascontiguousarray(
        (np.asarray(Wu, f32) @ np.asarray(bc, f32)).reshape(1, O))

    attn_s = attn * f32(1.0 / K)
    amean = attn_s.sum(axis=1, dtype=f32)
    # pre-scale contexts by attention weights (exact same multiply the
    # reference does); on-device the k-reduction becomes a pure sum
    ctx_s = contexts * attn_s[..., None]

    in_maps = []
    for c in range(NCORES):
        s, e = c * NPC, min((c + 1) * NPC, N)
        n = e - s
        xc = x[s:e]
        cc = ctx_s[s:e]
        mc = amean[s:e]
        if n < NPC:
            pad = NPC - n
            xc = np.concatenate([xc, np.zeros((pad, F), f32)])
            cc = np.concatenate([cc, np.zeros((pad, K, F), f32)])
            mc = np.concatenate([mc, np.zeros((pad,), f32)])
        in_maps.append({
            "ctxs": np.ascontiguousarray(cc),
            "xT": np.ascontiguousarray(xc.T),
            "amean": np.ascontiguousarray(mc.reshape(1, NPC)),
            "WnT": wn_h, "WcT": wc_h, "WuT": wu_h, "KbT": kb_h,
            "Wsp": wsp_h, "bn": bn_h, "bu": bu_h, "ybias": yb_h,
            "w2bc": w2bc_h,
        })

    nc = _get_nc()
    res = run_bass_kernel_spmd(nc, in_maps, list(range(NCORES)))
    last_results = res

    out = np.empty((N, O), f32)
    for c in range(NCORES):
        s, e = c * NPC, min((c + 1) * NPC, N)
        out[s:e] = res.results[c]["outT"].T[: e - s]
    return out


# revision 48
# speedup vs baseline: 4.9014x; 1.0315x over previous
"""Trainium2 Bass kernel for NodeEmbeddingLayer (gnn_message_passing).

Math (reference):
    xt = x @ Wn.T + bn                       # [N, H]
    ct = einsum('nkf,hf->nkh', ctx, Wc) + bc # [N, K, H]
    ca = (ct * attn[..,None]).mean(1)        # [N, H]
    u  = (xt + ca) @ Wu.T + bu               # [N, O]
    out = silu(u) @ Wb.T + einsum('nic,oic->no', bspline(u), Ws*scal)

Key rewrites (exact up to fp reassociation):
  1. Linearity of Wc:   ca = (mean_k attn_k*ctx_k) @ Wc.T + bc * (mean_k attn_k)
     -> the [N,K,F]x[H,F] einsum (84 GFLOP) becomes a [N,F] weighted
        reduction (cheap, DVE) + one [N,F]@[F,H] matmul.
  2. Quadratic B-splines on a uniform grid == alternating sums of shifted
     truncated powers relu(y-m)^2, y = 1.5*u + 3.5, m = 0..7:
        B2_c(y) = 0.5*(y-c)+^2 - 1.5*(y-c-1)+^2 + 1.5*(y-c-2)+^2 - 0.5*(y-c-3)+^2
     The (c->m) combination is linear, so it folds into the spline weights
     on the host:  spline = sum_{i,m} relu(y_i-m)^2 * Wt[o,i,m].

Device layout: everything runs transposed ([feature, node]) so that matmul
contractions chain without intermediate transposes; only cmean (produced
node-major by the attention reduction) is PE-transposed on chip. x is
pre-transposed on the host; the output is produced as outT [O, N] and
transposed back on the host.

Sharding: data-parallel over nodes, 2560 nodes/core x 8 cores (core 7 is
zero-padded 2080->2560).
"""

import numpy as np
from contextlib import ExitStack

import concourse.bass as bass
import concourse.tile as tile
from concourse import bacc, mybir, masks
from concourse.bass_utils import run_bass_kernel_spmd

N, K, F, H, O = 20000, 16, 256, 512, 256
NCORES = 8
NPC = 2560              # nodes per core (core 7 zero-padded)
GSZ = 512               # nodes per matmul group (moving free dim)
NG = NPC // GSZ         # 5 groups
NSUB = GSZ // 128       # 4 subtiles of 128 nodes per group
F32 = mybir.dt.float32

import os
USE_F32R = os.environ.get("K_F32R", "1") == "1"
MMDT = mybir.dt.float32r if USE_F32R else mybir.dt.float32

AF = mybir.ActivationFunctionType

# results of the last run_bass_kernel_spmd call (for test harness probing)
last_results = None


def _mm(ap):
    return ap.bitcast(MMDT)


def _build_nc():
    nc = bacc.Bacc("TRN2", target_bir_lowering=False, debug=False)

    ctx_d = nc.declare_dram_parameter("ctxs", [NPC, K, F], F32, isOutput=False)
    xT_d = nc.declare_dram_parameter("xT", [F, NPC], F32, isOutput=False)
    amean_d = nc.declare_dram_parameter("amean", [1, NPC], F32, isOutput=False)
    wn_d = nc.declare_dram_parameter("WnT", [F, H], F32, isOutput=False)
    wc_d = nc.declare_dram_parameter("WcT", [F, H], F32, isOutput=False)
    wu_d = nc.declare_dram_parameter("WuT", [H, O], F32, isOutput=False)
    kb_d = nc.declare_dram_parameter("KbT", [O, O], F32, isOutput=False)
    wsp_d = nc.declare_dram_parameter("Wsp", [16, 128, O], F32, isOutput=False)
    bn_d = nc.declare_dram_parameter("bn", [H], F32, isOutput=False)
    bu_d = nc.declare_dram_parameter("bu", [O], F32, isOutput=False)
    yb_d = nc.declare_dram_parameter("ybias", [O], F32, isOutput=False)
    w2bc_d = nc.declare_dram_parameter("w2bc", [1, O], F32, isOutput=False)
    out_d = nc.declare_dram_parameter("outT", [O, NPC], F32, isOutput=True)

    with tile.TileContext(nc) as tc, ExitStack() as ctx:
        const = ctx.enter_context(tc.tile_pool(name="const", bufs=1))
        ctxp = ctx.enter_context(tc.tile_pool(name="ctxp", bufs=5))
        cmtp = ctx.enter_context(tc.tile_pool(name="cmtp", bufs=2))
        vsbp = ctx.enter_context(tc.tile_pool(name="vsbp", bufs=6))
        actp = ctx.enter_context(tc.tile_pool(name="actp", bufs=3))
        rtp = ctx.enter_context(tc.tile_pool(name="rtp", bufs=2))
        rp = ctx.enter_context(tc.tile_pool(name="rp", bufs=4))
        outp = ctx.enter_context(tc.tile_pool(name="outp", bufs=2))
        ps_st = ctx.enter_context(tc.tile_pool(name="ps_st", bufs=2, space="PSUM"))
        ps_v = ctx.enter_context(tc.tile_pool(name="ps_v", bufs=2, space="PSUM"))
        ps_u = ctx.enter_context(tc.tile_pool(name="ps_u", bufs=2, space="PSUM"))
        ps_o = ctx.enter_context(tc.tile_pool(name="ps_o", bufs=2, space="PSUM"))

        # ---- constants / weights (loaded once) ----
        ident = const.tile([128, 128], F32)
        masks.make_identity(nc, ident[:])

        negm = const.tile([128, 8], F32)
        for m in range(8):
            nc.gpsimd.memset(negm[:, m:m + 1], float(-m))

        def load_rounded(shape, src_ap, name):
            """DMA fp32 into scratch (borrowing a ctx-pool slab), then
            round-copy into an MMDT tile so the tile has an explicit
            fp32r-rounding producer."""
            t = const.tile(shape, MMDT, name=name)
            if not USE_F32R:
                nc.gpsimd.dma_start(t[:], src_ap)
                return t
            free = 1
            for d in shape[1:]:
                free *= d
            s = ctxp.tile([shape[0], free], F32, tag="ctx", name=f"stg_{name}")
            nc.gpsimd.dma_start(s[:], src_ap)
            t_flat = t[:] if len(shape) == 2 else t[:].rearrange("p a b -> p (a b)")
            nc.vector.tensor_copy(t_flat, s[:])
            return t

        wn = load_rounded([128, 2, H],
                          wn_d.rearrange("(c p) h -> p c h", p=128), "wn")
        wc = load_rounded([128, 2, H],
                          wc_d.rearrange("(c p) h -> p c h", p=128), "wc")
        wu = load_rounded([128, 4, O],
                          wu_d.rearrange("(c p) o -> p c o", p=128), "wu")
        kb = load_rounded([128, 2, O],
                          kb_d.rearrange("(c p) o -> p c o", p=128), "kb")
        wsp = load_rounded([128, 16, O], wsp_d.rearrange("j p o -> p j o"), "wsp")
        bn = const.tile([128, 4], F32)
        nc.gpsimd.dma_start(bn[:], bn_d.rearrange("(c p) -> p c", p=128))
        bu = const.tile([128, 2], F32)
        nc.gpsimd.dma_start(bu[:], bu_d.rearrange("(c p) -> p c", p=128))
        yb = const.tile([128, 2], F32)
        nc.gpsimd.dma_start(yb[:], yb_d.rearrange("(c p) -> p c", p=128))
        w2bc = load_rounded([1, O], w2bc_d[:, :], "w2bc")

        xa = const.tile([128, 2, NPC], MMDT)
        if USE_F32R:
            for fc in range(2):
                sx = ctxp.tile([128, NPC], F32, tag="ctx", name=f"stg_x{fc}")
                nc.gpsimd.dma_start(sx[:], xT_d[fc * 128:(fc + 1) * 128, :])
                nc.vector.tensor_copy(xa[:, fc, :], sx[:])
        else:
            nc.gpsimd.dma_start(xa[:], xT_d.rearrange("(c p) n -> p c n", p=128))
        am = load_rounded([1, NPC], amean_d[:, :], "am")

        ctx_r = ctx_d.rearrange("(t p) k f -> t p k f", p=128)

        # last 512-node group split into two 256-node groups to halve
        # the pipeline drain tail
        groups = [(i * GSZ, GSZ) for i in range(NG - 1)]
        groups += [((NG - 1) * GSZ, GSZ // 2), ((NG - 1) * GSZ + GSZ // 2, GSZ // 2)]
        for g, (gs, gsz) in enumerate(groups):
            ge = gs + gsz
            nsub = gsz // 128

            # ---- context mean (ctx pre-scaled by attn on host) ----
            # cmeanT[f, n] = sum_k ctxs[n, k, f]: 16 accumulating PE
            # transposes straight into PSUM -> [F, nodes] with no DVE work.
            cmt = [cmtp.tile([128, gsz], F32, tag=f"cmT{fc}", name=f"cmt{fc}_{g}",
                             bufs=3)
                   for fc in range(2)]
            for t in range(nsub):
                nt = gs // 128 + t
                ct_t = ctxp.tile([128, K, F], F32, tag="ctx", name=f"ctx_{nt}")
                nc.sync.dma_start(ct_t[:], ctx_r[nt])
                # k in [KP, 16): DVE pairwise tree -> s3 [128, F]
                s1 = cmtp.tile([128, 4, F], F32, tag="tr1", name=f"tr1_{nt}")
                nc.vector.tensor_add(s1[:], ct_t[:, 8:12, :], ct_t[:, 12:16, :])
                s2 = cmtp.tile([128, 2, F], F32, tag="tr2", name=f"tr2_{nt}")
                nc.vector.tensor_add(s2[:], s1[:, 0:2, :], s1[:, 2:4, :])
                s3 = cmtp.tile([128, F], F32, tag="tr3", name=f"tr3_{nt}")
                nc.vector.tensor_add(s3[:], s2[:, 0, :], s2[:, 1, :])
                # k in [0, KP): accumulating PE transposes + one for s3
                for fc in range(2):
                    tp = ps_st.tile([128, 128], F32, tag="st", name=f"st{nt}_{fc}")
                    for k in range(8):
                        nc.tensor.matmul(
                            tp[:], ct_t[:, k, fc * 128:(fc + 1) * 128], ident[:],
                            is_transpose=True, start=(k == 0), stop=False)
                    nc.tensor.matmul(
                        tp[:], s3[:, fc * 128:(fc + 1) * 128], ident[:],
                        is_transpose=True, start=False, stop=True)
                    nc.vector.tensor_copy(
                        cmt[fc][:, t * 128:(t + 1) * 128].bitcast(MMDT), tp[:])

            # ---- stage 1: vT = Wn@xT + Wc@cmeanT (+bn via ACT bias) ----
            vsb = []
            for hc in range(4):
                vt = ps_v.tile([128, gsz], F32, tag="vT", name=f"vt{g}_{hc}")
                hs = hc * 128
                nc.tensor.matmul(vt[:], _mm(wn[:, 0, hs:hs + 128]),
                                 _mm(xa[:, 0, gs:ge]), start=True, stop=False)
                nc.tensor.matmul(vt[:], _mm(wn[:, 1, hs:hs + 128]),
                                 _mm(xa[:, 1, gs:ge]), start=False, stop=False)
                nc.tensor.matmul(vt[:], _mm(wc[:, 0, hs:hs + 128]),
                                 _mm(cmt[0][:]), start=False, stop=False)
                nc.tensor.matmul(vt[:], _mm(wc[:, 1, hs:hs + 128]),
                                 _mm(cmt[1][:]), start=False, stop=True)
                v = vsbp.tile([128, gsz], F32, tag="vsb", name=f"v{g}_{hc}")
                nc.scalar.activation(v[:].bitcast(MMDT), vt[:], AF.Identity,
                                     bias=bn[:, hc:hc + 1])
                vsb.append(v)

            # ---- stage 2: uT = Wu@vT + (Wu@bc) x amean ----
            silu, ysb = [], []
            for oc in range(2):
                ut = ps_u.tile([128, gsz], F32, tag="uT", name=f"ut{g}_{oc}")
                os_ = oc * 128
                for hc in range(4):
                    nc.tensor.matmul(ut[:], _mm(wu[:, hc, os_:os_ + 128]),
                                     _mm(vsb[hc][:]), start=(hc == 0), stop=False)
                nc.tensor.matmul(ut[:], _mm(w2bc[0:1, os_:os_ + 128]),
                                 _mm(am[0:1, gs:ge]), start=False, stop=True)
                s = actp.tile([128, gsz], F32, tag="silu", name=f"s{g}_{oc}")
                nc.scalar.activation(s[:].bitcast(MMDT), ut[:], AF.Silu,
                                     bias=bu[:, oc:oc + 1])
                y = actp.tile([128, gsz], F32, tag="y", name=f"y{g}_{oc}")
                nc.scalar.activation(y[:], ut[:], AF.Identity,
                                     bias=yb[:, oc:oc + 1], scale=1.5)
                silu.append(s)
                ysb.append(y)

            # ---- stage 3: outT = Kb@silu + Wsp@relu(y-m)^2 ----
            ot = [ps_o.tile([128, gsz], F32, tag="outT", name=f"ot{g}_{oc}")
                  for oc in range(2)]
            for oc in range(2):
                os_ = oc * 128
                for ic in range(2):
                    nc.tensor.matmul(ot[oc][:], _mm(kb[:, ic, os_:os_ + 128]),
                                     _mm(silu[ic][:]), start=(ic == 0), stop=False)
            for m in range(8):
                for ic in range(2):
                    # relu(y - m) then square; spread across ACT/DVE/GpSimd
                    idx = m * 2 + ic
                    rt = rtp.tile([128, gsz], F32, tag="rt", name=f"rt{g}_{m}_{ic}")
                    if idx < 8:
                        nc.scalar.activation(rt[:], ysb[ic][:], AF.Relu,
                                             bias=negm[:, m:m + 1])
                    else:
                        nc.vector.tensor_scalar(
                            rt[:], ysb[ic][:], float(-m), 0.0,
                            mybir.AluOpType.add, mybir.AluOpType.max)
                    r = rp.tile([128, gsz], F32, tag="r", name=f"r{g}_{m}_{ic}")
                    if idx < 6:
                        nc.vector.tensor_mul(r[:].bitcast(MMDT), rt[:], rt[:])
                    else:
                        nc.gpsimd.tensor_mul(r[:].bitcast(MMDT), rt[:], rt[:])
                    j = m * 2 + ic
                    last = (m == 7 and ic == 1)
                    for oc in range(2):
                        os_ = oc * 128
                        nc.tensor.matmul(ot[oc][:], _mm(wsp[:, j, os_:os_ + 128]),
                                         _mm(r[:]), start=False, stop=last)
            for oc in range(2):
                osb = outp.tile([128, gsz], F32, tag="osb", name=f"osb{g}_{oc}")
                nc.vector.tensor_copy(osb[:], ot[oc][:])
                # separate DMA queue (SWDGE) so output stores never
                # head-of-line block the in-order ctx load stream on SP
                nc.gpsimd.dma_start(out_d[oc * 128:(oc + 1) * 128, gs:ge], osb[:])

    nc.finalize()
    return nc


_nc_cache = None


def _get_nc():
    global _nc_cache
    if _nc_cache is None:
        _nc_cache = _build_nc()
    return _nc_cache


def kernel(x, contexts, attn, Wn, bn, Wc, bc, Wu, bu, kan_base_w,
           kan_spline_w, kan_scaler):
    global last_results
    f32 = np.float32

    x = np.asarray(x, f32)
    contexts = np.asarray(contexts, f32)
    attn = np.asarray(attn, f32)

    # fold spline basis combination into the weights:
    # B2_c(y) = sum_t alpha[t] * relu(y - (c+t))^2, alpha = [.5,-1.5,1.5,-.5]
    ws = np.asarray(kan_spline_w, f32) * np.asarray(kan_scaler, f32)[..., None]
    alpha = np.array([0.5, -1.5, 1.5, -0.5], f32)
    wt = np.zeros((O, O, 8), f32)  # [o, i, m]
    for c in range(5):
        for t in range(4):
            wt[:, :, c + t] += alpha[t] * ws[:, :, c]
    # [(m, i), o] chunked for the PE: j = m*2 + ic, row p = i - ic*128
    wsp_h = np.ascontiguousarray(
        wt.transpose(2, 1, 0).reshape(16, 128, O))

    wn_h = np.ascontiguousarray(np.asarray(Wn, f32).T)       # [F, H]
    wc_h = np.ascontiguousarray(np.asarray(Wc, f32).T)       # [F, H]
    wu_h = np.ascontiguousarray(np.asarray(Wu, f32).T)       # [H, O]
    kb_h = np.ascontiguousarray(np.asarray(kan_base_w, f32).T)  # [O, O]
    bn_h = np.ascontiguousarray(np.asarray(bn, f32))
    bu_h = np.ascontiguousarray(np.asarray(bu, f32))
    yb_h = np.ascontiguousarray(bu_h * f32(1.5) + f32(3.5))
    w2bc_h = np.ascontiguousarray(
        (np.asarray(Wu, f32) @ np.asarray(bc, f32)).reshape(1, O))

    attn_s = attn * f32(1.0 / K)
    amean = attn_s.sum(axis=1, dtype=f32)
    # pre-scale contexts by attention weights (exact same multiply the
    # reference does); on-device the k-reduction becomes a pure sum
    ctx_s = contexts * attn_s[..., None]

    in_maps = []
    for c in range(NCORES):
        s, e = c * NPC, min((c + 1) * NPC, N)
        n = e - s
        xc = x[s:e]
        cc = ctx_s[s:e]
        mc = amean[s:e]
        if n < NPC:
            pad = NPC - n
            xc = np.concatenate([xc, np.zeros((pad, F), f32)])
            cc = np.concatenate([cc, np.zeros((pad, K, F), f32)])
            mc = np.concatenate([mc, np.zeros((pad,), f32)])
        in_maps.append({
            "ctxs": np.ascontiguousarray(cc),
            "xT": np.ascontiguousarray(xc.T),
            "amean": np.ascontiguousarray(mc.reshape(1, NPC)),
            "WnT": wn_h, "WcT": wc_h, "WuT": wu_h, "KbT": kb_h,
            "Wsp": wsp_h, "bn": bn_h, "bu": bu_h, "ybias": yb_h,
            "w2bc": w2bc_h,
        })

    nc = _get_nc()
    res = run_bass_kernel_spmd(nc, in_maps, list(range(NCORES)))
    last_results = res

    out = np.empty((N, O), f32)
    for c in range(NCORES):
        s, e = c * NPC, min((c + 1) * NPC, N)
        out[s:e] = res.results[c]["outT"].T[: e - s]
    return out
